# revision 2
# baseline (speedup 1.0000x reference)
"""Trainium2 Bass kernel for nn_CrossAttention (B=4, C=256, N=64*64=4096, CQK=32).

Reference computation:
    q = Wq @ xf + bq          [B, N, 32]
    k = Wk @ yf + bk          [B, 32, N]
    v = Wv @ yf + bv          [B, 256, N]
    attn = softmax(q @ k)     [B, N, N]
    out = gamma * (v @ attn^T) + x

Sharding: 8 cores = batch(4) x query-half(2). Each core owns 2048 query
positions of one sample and all 4096 keys of that sample.

v2 design notes (vs the earlier baseline):
  - No zero-padding of q/k to 128 partitions. Energy matmuls run at K=32
    with 4-way row tiling (tile_position=(32i,0)): 4 key chunks execute
    concurrently in the PE array. qT is replicated 4x along partitions via
    column-tiled projection matmuls; kT lives as kT2[32i, s, :] so each
    chunk's stationary sits at its row-group's partitions.
  - exp granularity 1024 (one ACT instruction per key-chunk pair, reading a
    2-bank PSUM tile [128,2,512]) - the scalar engine is the bottleneck, and
    its ~150ns per-instruction overhead is paid half as often.
  - AV uses fp8 DoubleRow exactly as before (ex [128,2,512] -> pouts[q,VW]
    with a ones-column producing the softmax denominator in col 256).
  - Output is written q-major ([qchunk,128,C] bf16); the host transposes
    back to [C, N] fp32. This removes all PE transposes from the kernel.
  - bv is folded into vaug (broadcast row tile), biases bq/bk are folded in
    the projection PSUM->SBUF moves with 4x-replicated bias vectors.
  - vaug build + kT2/qT projections are interleaved into quarter 0 so the
    PE prologue overlaps the ACT-bound attention stream.
  - DMA only on the sync + gpsimd queues (the scalar queue must stay free
    for the 128 exp activations; a dummy exp warms the ACT table early).
"""

import contextlib

import numpy as np

import concourse.mybir as mybir
import concourse.tile as tile
from concourse import bacc
from concourse.bass_utils import run_bass_kernel_spmd

F32 = mybir.dt.float32
F8 = mybir.dt.float8e4
BF16 = mybir.dt.bfloat16
AFT = mybir.ActivationFunctionType
DR = mybir.MatmulPerfMode.DoubleRow

B = 4
C = 256
CQK = 32
N = 4096  # 64 * 64
NCORES = 8
NLOC = N // 2  # 2048 queries per core
CCH = C // 128  # 2 channel chunks
MC = N // 128  # 32 key chunks
NPAIR = MC // 2  # 16 key-chunk pairs
NQ = 4  # query quarters per core
QW = NLOC // NQ  # 512
NQC = NLOC // 128  # 16 query chunks of 128
VW = 272  # vaug width: 256 v channels + denominator col + pad to step%16==0


def _trace_kernel(ctx, tc, y_d, xb_d, xq_d, wq_d, wk_d, wv_d, bq_d, bk_d, bvb_d, g_d, out_d):
    nc = tc.nc

    const = ctx.enter_context(tc.tile_pool(name="const", bufs=1))
    big = ctx.enter_context(tc.tile_pool(name="big", bufs=1))
    vaugp = ctx.enter_context(tc.tile_pool(name="vaugp", bufs=NPAIR))
    expp = ctx.enter_context(tc.tile_pool(name="expp", bufs=3))
    finp = ctx.enter_context(tc.tile_pool(name="finp", bufs=4))
    smallp = ctx.enter_context(tc.tile_pool(name="smallp", bufs=8))
    # PSUM (8 banks): pep = 2 x [128,2,512] f32 (2 banks each, shared by the
    # energy pairs and, as rotation passengers, the projection matmuls);
    # poutp = 4 x [128,272] f32 (1 bank each) AV accumulators.
    pep = ctx.enter_context(tc.tile_pool(name="pep", bufs=2, space="PSUM"))
    poutp = ctx.enter_context(tc.tile_pool(name="poutp", bufs=4, space="PSUM"))

    # ---- tiny consts + ACT exp table warm-up (before anything else on ACT)
    scr = const.tile([1, 1], F32, tag="scr")
    nc.vector.memset(scr, 0.0)
    nc.scalar.activation(scr, scr, AFT.Exp)

    onep_sb = const.tile([128, VW - C], F8, tag="onep_sb")
    nc.vector.memset(onep_sb, 0.0)
    nc.vector.memset(onep_sb[:, 0:1], 1.0)

    # ---- weight/const DMAs on the sync queue (first: they gate projections)
    wk_b = const.tile([128, CCH, CQK], BF16, tag="wk_b")
    nc.sync.dma_start(out=wk_b, in_=wk_d.ap())
    wq_b = const.tile([128, CCH, CQK], BF16, tag="wq_b")
    nc.sync.dma_start(out=wq_b, in_=wq_d.ap())
    bq_sb = const.tile([128, 1], F32, tag="bq_sb")
    nc.sync.dma_start(out=bq_sb, in_=bq_d.ap())
    bk_sb = const.tile([128, 1], F32, tag="bk_sb")
    nc.sync.dma_start(out=bk_sb, in_=bk_d.ap())
    g_sb = const.tile([128, 1], F32, tag="g_sb")
    nc.sync.dma_start(out=g_sb, in_=g_d.ap())
    wv_b = const.tile([128, CCH, C], BF16, tag="wv_b")
    nc.sync.dma_start(out=wv_b, in_=wv_d.ap())
    bvb_sb = const.tile([128, C], BF16, tag="bvb_sb")
    nc.sync.dma_start(out=bvb_sb, in_=bvb_d.ap())

    # ---- activation DMAs: y on sync (16 pieces, gates kT2+vaug); x_bf and
    # x_q on the gpsimd SWDGE queue (gpsimd is otherwise idle).
    y_sb = big.tile([128, CCH, N], BF16, tag="y_sb")
    for s in range(8):
        sl = slice(s * QW, (s + 1) * QW)
        for cc in range(CCH):
            nc.sync.dma_start(out=y_sb[:, cc, sl], in_=y_d.ap()[cc, :, sl])
    xb_sb = big.tile([128, CCH, NLOC], BF16, tag="xb_sb")
    for s in range(4):
        sl = slice(s * QW, (s + 1) * QW)
        for cc in range(CCH):
            nc.gpsimd.dma_start(out=xb_sb[:, cc, sl], in_=xb_d.ap()[cc, :, sl])
    xq_sb = big.tile([128, NQC, C], F32, tag="xq_sb")
    for k in range(NQC):
        nc.gpsimd.dma_start(out=xq_sb[:, k, :], in_=xq_d.ap()[k])

    kT2_sb = big.tile([128, 8, 128], BF16, tag="kT2_sb")
    qTr = big.tile([128, NLOC], BF16, tag="qTr")

    # ---- projections (column-tiled: out partitions 32i via tile_position) --
    def kT2_slice(s):
        # keys [512s, 512s+512) = chunks 4s+i, chunk 4s+i -> partitions 32i
        pkt = pep.tile([128, 128], F32, tag="pe", name=f"pkt{s}")
        for cc in range(CCH):
            for i in range(4):
                c = 4 * s + i
                nc.tensor.matmul(
                    pkt[32 * i : 32 * i + 32, :],
                    lhsT=wk_b[:, cc, :],
                    rhs=y_sb[:, cc, c * 128 : (c + 1) * 128],
                    start=(cc == 0),
                    stop=(cc == CCH - 1),
                    tile_position=(0, 32 * i),
                )
        nc.vector.tensor_scalar_add(kT2_sb[:, s, :], pkt, bk_sb)

    def qT_slice(s):
        # queries [512s, 512s+512), q replicated to all 4 row groups
        pqt = pep.tile([128, QW], F32, tag="pe", name=f"pqt{s}")
        for cc in range(CCH):
            for i in range(4):
                nc.tensor.matmul(
                    pqt[32 * i : 32 * i + 32, :],
                    lhsT=wq_b[:, cc, :],
                    rhs=xb_sb[:, cc, s * QW : (s + 1) * QW],
                    start=(cc == 0),
                    stop=(cc == CCH - 1),
                    tile_position=(0, 32 * i),
                )
        nc.vector.tensor_scalar_add(qTr[:, s * QW : (s + 1) * QW], pqt, bq_sb)

    vaug = []

    def vaug_build(g):
        va = vaugp.tile([128, 2, VW], F8, tag="vaug", name=f"vaug{g}")
        for j in range(2):
            mc = 2 * g + j
            pv = pep.tile([128, C], F32, tag="pe", name=f"pv{mc}")
            for cc in range(CCH):
                nc.tensor.matmul(
                    pv,
                    lhsT=y_sb[:, cc, mc * 128 : (mc + 1) * 128],
                    rhs=wv_b[:, cc, :],
                    start=(cc == 0),
                    stop=(cc == CCH - 1),
                )
            nc.vector.tensor_add(va[:, j, :C], pv, bvb_sb)
            nc.vector.tensor_copy(va[:, j, C:VW], onep_sb)
        vaug.append(va)

    kT2_slice(0)
    kT2_slice(1)
    qT_slice(0)

    # ---- attention quarters --------------------------------------------
    for qt in range(NQ):
        nsl = slice(qt * QW, (qt + 1) * QW)
        pouts = [
            poutp.tile([128, VW], F32, tag="pout", name=f"pout{qt}_{i}")
            for i in range(4)
        ]

        def do_av(g, ex):
            for ncc in range(4):
                nc.tensor.matmul(
                    pouts[ncc],
                    lhsT=ex[:, :, ncc * 128 : (ncc + 1) * 128],
                    rhs=vaug[g],
                    start=(g == 0),
                    stop=(g == NPAIR - 1),
                    perf_mode=DR,
                )

        prev = None
        for g in range(NPAIR):
            if qt == 0:
                if g % 2 == 0 and g // 2 + 2 <= 7:
                    kT2_slice(g // 2 + 2)
                vaug_build(g)
            pe_t = pep.tile([128, 2, QW], F32, tag="pe", name=f"pe{qt}_{g}")
            for j in range(2):
                c = 2 * g + j
                pos = 32 * (c % 4)
                nc.tensor.matmul(
                    pe_t[:, j, :],
                    lhsT=kT2_sb[pos : pos + 32, c // 4, :],
                    rhs=qTr[pos : pos + 32, nsl],
                    start=True,
                    stop=True,
                    tile_position=(pos, 0),
                )
            ex = expp.tile([128, 2, QW], F8, tag="exp", name=f"ex{qt}_{g}")
            nc.scalar.activation(ex, pe_t, AFT.Exp)
            if prev is not None:
                do_av(*prev)
            prev = (g, ex)
        do_av(*prev)

        if qt + 1 < NQ:
            qT_slice(qt + 1)

        # drain: normalize by the denominator column, scale by gamma, add the
        # (host-pretransposed) residual, DMA out q-major bf16.
        for ncc in range(4):
            po = pouts[ncc]
            qidx = qt * 4 + ncc
            rec = smallp.tile([128, 1], F32, tag="rec", name=f"rec{qidx}")
            nc.vector.reciprocal(rec, po[:, C : C + 1])
            grec = smallp.tile([128, 1], F32, tag="grec", name=f"grec{qidx}")
            nc.vector.tensor_scalar_mul(grec, rec, g_sb)
            tmp = finp.tile([128, C], BF16, tag="tmp", name=f"tmp{qidx}")
            nc.vector.tensor_scalar_mul(tmp, po[:, :C], grec)
            fin = finp.tile([128, C], BF16, tag="fin", name=f"fin{qidx}")
            nc.vector.tensor_add(fin, tmp, xq_sb[:, qidx, :])
            nc.sync.dma_start(out=out_d.ap()[qidx], in_=fin)


_PROGRAM_CACHE = {}


def _get_program():
    if "nc" in _PROGRAM_CACHE:
        return _PROGRAM_CACHE["nc"]
    nc = bacc.Bacc("TRN2", target_bir_lowering=False, debug=False)
    y_d = nc.dram_tensor("y2", [CCH, 128, N], BF16, kind="ExternalInput")
    xb_d = nc.dram_tensor("xb", [CCH, 128, NLOC], BF16, kind="ExternalInput")
    xq_d = nc.dram_tensor("xq", [NQC, 128, C], F32, kind="ExternalInput")
    wq_d = nc.dram_tensor("wq_t", [128, CCH, CQK], BF16, kind="ExternalInput")
    wk_d = nc.dram_tensor("wk_t", [128, CCH, CQK], BF16, kind="ExternalInput")
    wv_d = nc.dram_tensor("wv_t", [128, CCH, C], BF16, kind="ExternalInput")
    bq_d = nc.dram_tensor("bq_rep", [128, 1], F32, kind="ExternalInput")
    bk_d = nc.dram_tensor("bk_rep", [128, 1], F32, kind="ExternalInput")
    bvb_d = nc.dram_tensor("bv_bc", [128, C], BF16, kind="ExternalInput")
    g_d = nc.dram_tensor("gamma_b", [128, 1], F32, kind="ExternalInput")
    out_d = nc.dram_tensor("out_q", [NQC, 128, C], BF16, kind="ExternalOutput")
    with tile.TileContext(nc) as tc, contextlib.ExitStack() as ctx:
        _trace_kernel(
            ctx, tc, y_d, xb_d, xq_d, wq_d, wk_d, wv_d, bq_d, bk_d, bvb_d, g_d, out_d
        )
    nc.compile()
    _PROGRAM_CACHE["nc"] = nc
    return nc


def _make_in_maps(inputs):
    import ml_dtypes

    BF = ml_dtypes.bfloat16
    x = np.ascontiguousarray(inputs["x"], dtype=np.float32).reshape(B, C, N)
    y = np.asarray(inputs["y"], np.float32).reshape(B, C, N)
    y_bf = np.ascontiguousarray(y.astype(BF))
    wq_t = np.ascontiguousarray(
        np.asarray(inputs["Wq"], np.float32)
        .astype(BF).T.reshape(CCH, 128, CQK).transpose(1, 0, 2)
    )
    wk_t = np.ascontiguousarray(
        np.asarray(inputs["Wk"], np.float32)
        .astype(BF).T.reshape(CCH, 128, CQK).transpose(1, 0, 2)
    )
    wv_t = np.ascontiguousarray(
        np.asarray(inputs["Wv"], np.float32)
        .astype(BF).T.reshape(CCH, 128, C).transpose(1, 0, 2)
    )
    bq = np.asarray(inputs["bq"], np.float32).reshape(CQK)
    bk = np.asarray(inputs["bk"], np.float32).reshape(CQK)
    bq_rep = np.ascontiguousarray(np.tile(bq, 4).reshape(128, 1))
    bk_rep = np.ascontiguousarray(np.tile(bk, 4).reshape(128, 1))
    bv_bc = np.ascontiguousarray(
        np.broadcast_to(
            np.asarray(inputs["bv"], np.float32).reshape(1, C), (128, C)
        ).astype(BF)
    )
    gamma_b = np.full(
        (128, 1), float(np.asarray(inputs["gamma"]).reshape(-1)[0]), np.float32
    )

    in_maps = []
    for core in range(NCORES):
        b, h = divmod(core, 2)
        xl = x[b, :, h * NLOC : (h + 1) * NLOC]  # [C, NLOC]
        xb = np.ascontiguousarray(xl.astype(BF).reshape(CCH, 128, NLOC))
        xq = np.ascontiguousarray(xl.T.reshape(NQC, 128, C))
        in_maps.append(
            {
                "y2": np.ascontiguousarray(y_bf[b].reshape(CCH, 128, N)),
                "xb": xb,
                "xq": xq,
                "wq_t": wq_t,
                "wk_t": wk_t,
                "wv_t": wv_t,
                "bq_rep": bq_rep,
                "bk_rep": bk_rep,
                "bv_bc": bv_bc,
                "gamma_b": gamma_b,
            }
        )
    return in_maps


def _assemble(results):
    out = np.empty((B, C, N), np.float32)
    for core in range(NCORES):
        b, h = divmod(core, 2)
        oq = np.asarray(results[core]["out_q"], dtype=np.float32)  # [16,128,C]
        out[b, :, h * NLOC : (h + 1) * NLOC] = oq.reshape(NLOC, C).T
    return out.reshape(B, C, 64, 64)


def run(inputs, trace=False, **kwargs):
    """Run the kernel; returns (full_output, BassKernelResults)."""
    nc = _get_program()
    in_maps = _make_in_maps(inputs)
    res = run_bass_kernel_spmd(
        nc, in_maps, core_ids=list(range(NCORES)), trace=trace, **kwargs
    )
    return _assemble(res.results), res


def kernel(**inputs) -> np.ndarray:
    out, _ = run(inputs, trace=False)
    return out


# revision 4
# speedup vs baseline: 1.3053x; 1.3053x over previous
"""Trainium2 Bass kernel for nn_CrossAttention (B=4, C=256, N=64*64=4096, CQK=32).

Reference computation:
    q = Wq @ xf + bq          [B, N, 32]
    k = Wk @ yf + bk          [B, 32, N]
    v = Wv @ yf + bv          [B, 256, N]
    attn = softmax(q @ k)     [B, N, N]
    out = gamma * (v @ attn^T) + x

Sharding: 8 cores = batch(4) x query-half(2). Each core owns 2048 query
positions of one sample and all 4096 keys of that sample.

v3 design notes:
  - Energy matmuls are full-array K=128 (kT/qT zero-padded 32->128).
    Row/col-masked tile_position matmuls are NOT counted as busy by the
    PE HAM activity monitor, which re-throttles the clock to 1.2GHz (v2
    measured 51% of the kernel at K=4/8) - so no PE tiling tricks.
    The zero rows are written once by two vector-engine memsets in the
    prologue (the old gpsimd memsets took 10us and gated the first MM).
  - exp granularity 1024: one ACT instruction per key-chunk pair reads a
    2-bank PSUM tile [128,2,512] and writes the fp8 ex tile. The scalar
    engine is the kernel's bottleneck; its ~150ns per-instruction overhead
    is paid half as often as with 512-wide activations.
  - AV uses fp8 DoubleRow (ex [128,2,512] stationary slices, vaug moving,
    pouts [q,272] with a ones-column producing the softmax denominator in
    col 256). bv is folded into vaug via a broadcast row tile.
  - Output is written q-major ([qchunk,128,C] bf16); the host transposes
    back to [C, N] fp32. This removes all PE transposes from the kernel.
  - vaug pairs 8-15 + kT slices 3-7 build inside quarter 0 so the PE
    prologue overlaps the ACT-bound attention stream.
  - DMA only on the sync + gpsimd queues (the scalar queue must stay free
    for the exp activations; a dummy exp warms the ACT table early).
"""

import contextlib

import numpy as np

import concourse.mybir as mybir
import concourse.tile as tile
from concourse import bacc
from concourse.bass_utils import run_bass_kernel_spmd

F32 = mybir.dt.float32
F8 = mybir.dt.float8e4
BF16 = mybir.dt.bfloat16
AFT = mybir.ActivationFunctionType
DR = mybir.MatmulPerfMode.DoubleRow

B = 4
C = 256
CQK = 32
N = 4096  # 64 * 64
NCORES = 8
NLOC = N // 2  # 2048 queries per core
CCH = C // 128  # 2 channel chunks
MC = N // 128  # 32 key chunks
NPAIR = MC // 2  # 16 key-chunk pairs
NQ = 4  # query quarters per core
QW = NLOC // NQ  # 512
NQC = NLOC // 128  # 16 query chunks of 128
VW = 272  # vaug width: 256 v channels + denominator col + pad to step%16==0


def _trace_kernel(ctx, tc, y_d, xb_d, xq_d, wq_d, wk_d, wv_d, bq_d, bk_d, bvb_d, g_d, out_d):
    nc = tc.nc

    const = ctx.enter_context(tc.tile_pool(name="const", bufs=1))
    big = ctx.enter_context(tc.tile_pool(name="big", bufs=1))
    vaugp = ctx.enter_context(tc.tile_pool(name="vaugp", bufs=NPAIR))
    expp = ctx.enter_context(tc.tile_pool(name="expp", bufs=3))
    finp = ctx.enter_context(tc.tile_pool(name="finp", bufs=4))
    smallp = ctx.enter_context(tc.tile_pool(name="smallp", bufs=8))
    # PSUM (8 banks): pep = 2 x [128,2,512] f32 (2 banks each, shared by the
    # energy pairs and, as rotation passengers, the projection matmuls);
    # poutp = 4 x [128,272] f32 (1 bank each) AV accumulators.
    pep = ctx.enter_context(tc.tile_pool(name="pep", bufs=2, space="PSUM"))
    poutp = ctx.enter_context(tc.tile_pool(name="poutp", bufs=4, space="PSUM"))

    # ---- tiny consts + ACT exp table warm-up (before anything else on ACT)
    scr = const.tile([1, 1], F32, tag="scr")
    nc.vector.memset(scr, 0.0)
    nc.scalar.activation(scr, scr, AFT.Exp)

    onep_sb = const.tile([128, VW - C], F8, tag="onep_sb")
    nc.vector.memset(onep_sb, 0.0)
    nc.vector.memset(onep_sb[:, 0:1], 1.0)

    # padded projection outputs; rows 32:128 stay zero so the energy matmul
    # can contract over the full 128 partitions (full-array MMs keep HAM warm
    # and get the fast weight load).
    kT_sb = big.tile([128, N], BF16, tag="kT_sb")
    nc.vector.memset(kT_sb, 0.0)
    qT_sb = big.tile([128, NLOC], BF16, tag="qT_sb")
    nc.vector.memset(qT_sb, 0.0)

    # ---- weight/const DMAs on the sync queue (first: they gate projections)
    wk_b = const.tile([128, CCH, CQK], BF16, tag="wk_b")
    nc.sync.dma_start(out=wk_b, in_=wk_d.ap())
    wq_b = const.tile([128, CCH, CQK], BF16, tag="wq_b")
    nc.sync.dma_start(out=wq_b, in_=wq_d.ap())
    bq_sb = const.tile([CQK, 1], F32, tag="bq_sb")
    nc.sync.dma_start(out=bq_sb, in_=bq_d.ap())
    bk_sb = const.tile([CQK, 1], F32, tag="bk_sb")
    nc.sync.dma_start(out=bk_sb, in_=bk_d.ap())
    g_sb = const.tile([128, 1], F32, tag="g_sb")
    nc.sync.dma_start(out=g_sb, in_=g_d.ap())
    wv_b = const.tile([128, CCH, C], BF16, tag="wv_b")
    nc.sync.dma_start(out=wv_b, in_=wv_d.ap())
    bvb_sb = const.tile([128, C], BF16, tag="bvb_sb")
    nc.sync.dma_start(out=bvb_sb, in_=bvb_d.ap())

    # ---- activation DMAs: y on sync (16 pieces, gates kT+vaug); x_bf and
    # x_q on the gpsimd SWDGE queue (gpsimd is otherwise idle).
    y_sb = big.tile([128, CCH, N], BF16, tag="y_sb")
    for s in range(8):
        sl = slice(s * QW, (s + 1) * QW)
        for cc in range(CCH):
            nc.sync.dma_start(out=y_sb[:, cc, sl], in_=y_d.ap()[cc, :, sl])
    xb_sb = big.tile([128, CCH, NLOC], BF16, tag="xb_sb")
    for s in range(4):
        sl = slice(s * QW, (s + 1) * QW)
        for cc in range(CCH):
            nc.gpsimd.dma_start(out=xb_sb[:, cc, sl], in_=xb_d.ap()[cc, :, sl])
    xq_sb = big.tile([128, NQC, C], F32, tag="xq_sb")
    for k in range(NQC):
        nc.gpsimd.dma_start(out=xq_sb[:, k, :], in_=xq_d.ap()[k])

    # ---- projections (baseline-style full-contraction matmuls) -----------
    def kT_slice(s):
        # keys [512s, 512s+512)
        sl = slice(s * QW, (s + 1) * QW)
        pk = pep.tile([CQK, QW], F32, tag="pe", name=f"pk{s}")
        for cc in range(CCH):
            nc.tensor.matmul(
                pk,
                lhsT=wk_b[:, cc, :],
                rhs=y_sb[:, cc, sl],
                start=(cc == 0),
                stop=(cc == CCH - 1),
            )
        nc.vector.tensor_scalar_add(kT_sb[0:CQK, sl], pk, bk_sb)

    def qT_slice(s):
        sl = slice(s * QW, (s + 1) * QW)
        pq = pep.tile([CQK, QW], F32, tag="pe", name=f"pq{s}")
        for cc in range(CCH):
            nc.tensor.matmul(
                pq,
                lhsT=wq_b[:, cc, :],
                rhs=xb_sb[:, cc, sl],
                start=(cc == 0),
                stop=(cc == CCH - 1),
            )
        nc.vector.tensor_scalar_add(qT_sb[0:CQK, sl], pq, bq_sb)

    vaug = []

    def vaug_build(g):
        va = vaugp.tile([128, 2, VW], F8, tag="vaug", name=f"vaug{g}")
        for j in range(2):
            mc = 2 * g + j
            pv = pep.tile([128, C], F32, tag="pe", name=f"pv{mc}")
            for cc in range(CCH):
                nc.tensor.matmul(
                    pv,
                    lhsT=y_sb[:, cc, mc * 128 : (mc + 1) * 128],
                    rhs=wv_b[:, cc, :],
                    start=(cc == 0),
                    stop=(cc == CCH - 1),
                )
            nc.vector.tensor_add(va[:, j, :C], pv, bvb_sb)
            nc.vector.tensor_copy(va[:, j, C:VW], onep_sb)
        vaug.append(va)

    # prologue: just enough to start the exp stream, plus the full qT (xb
    # arrives early on the gpsimd queue) and the first half of vaug.
    kT_slice(0)
    qT_slice(0)
    kT_slice(1)
    kT_slice(2)
    for s in range(1, NQ):
        qT_slice(s)
    for g in range(8):
        vaug_build(g)

    # ---- attention quarters --------------------------------------------
    for qt in range(NQ):
        nsl = slice(qt * QW, (qt + 1) * QW)
        pouts = [
            poutp.tile([128, VW], F32, tag="pout", name=f"pout{qt}_{i}")
            for i in range(4)
        ]

        def do_av(g, ex):
            for ncc in range(4):
                nc.tensor.matmul(
                    pouts[ncc],
                    lhsT=ex[:, :, ncc * 128 : (ncc + 1) * 128],
                    rhs=vaug[g],
                    start=(g == 0),
                    stop=(g == NPAIR - 1),
                    perf_mode=DR,
                )

        prev = None
        for g in range(NPAIR):
            if qt == 0:
                if g % 2 == 0 and g // 2 + 3 <= 7:
                    kT_slice(g // 2 + 3)
                if g < 8:
                    vaug_build(g + 8)
            pe_t = pep.tile([128, 2, QW], F32, tag="pe", name=f"pe{qt}_{g}")
            for j in range(2):
                c = 2 * g + j
                nc.tensor.matmul(
                    pe_t[:, j, :],
                    lhsT=kT_sb[:, c * 128 : (c + 1) * 128],
                    rhs=qT_sb[:, nsl],
                    start=True,
                    stop=True,
                )
            ex = expp.tile([128, 2, QW], F8, tag="exp", name=f"ex{qt}_{g}")
            nc.scalar.activation(ex, pe_t, AFT.Exp)
            if prev is not None:
                do_av(*prev)
            prev = (g, ex)
        do_av(*prev)

        # drain: normalize by the denominator column, scale by gamma, add the
        # (host-pretransposed) residual, DMA out q-major bf16.
        for ncc in range(4):
            po = pouts[ncc]
            qidx = qt * 4 + ncc
            rec = smallp.tile([128, 1], F32, tag="rec", name=f"rec{qidx}")
            nc.vector.reciprocal(rec, po[:, C : C + 1])
            grec = smallp.tile([128, 1], F32, tag="grec", name=f"grec{qidx}")
            nc.vector.tensor_scalar_mul(grec, rec, g_sb)
            tmp = finp.tile([128, C], BF16, tag="tmp", name=f"tmp{qidx}")
            nc.vector.tensor_scalar_mul(tmp, po[:, :C], grec)
            fin = finp.tile([128, C], BF16, tag="fin", name=f"fin{qidx}")
            nc.vector.tensor_add(fin, tmp, xq_sb[:, qidx, :])
            nc.sync.dma_start(out=out_d.ap()[qidx], in_=fin)


_PROGRAM_CACHE = {}


def _get_program():
    if "nc" in _PROGRAM_CACHE:
        return _PROGRAM_CACHE["nc"]
    nc = bacc.Bacc("TRN2", target_bir_lowering=False, debug=False)
    y_d = nc.dram_tensor("y2", [CCH, 128, N], BF16, kind="ExternalInput")
    xb_d = nc.dram_tensor("xb", [CCH, 128, NLOC], BF16, kind="ExternalInput")
    xq_d = nc.dram_tensor("xq", [NQC, 128, C], F32, kind="ExternalInput")
    wq_d = nc.dram_tensor("wq_t", [128, CCH, CQK], BF16, kind="ExternalInput")
    wk_d = nc.dram_tensor("wk_t", [128, CCH, CQK], BF16, kind="ExternalInput")
    wv_d = nc.dram_tensor("wv_t", [128, CCH, C], BF16, kind="ExternalInput")
    bq_d = nc.dram_tensor("bq_c", [CQK, 1], F32, kind="ExternalInput")
    bk_d = nc.dram_tensor("bk_c", [CQK, 1], F32, kind="ExternalInput")
    bvb_d = nc.dram_tensor("bv_bc", [128, C], BF16, kind="ExternalInput")
    g_d = nc.dram_tensor("gamma_b", [128, 1], F32, kind="ExternalInput")
    out_d = nc.dram_tensor("out_q", [NQC, 128, C], BF16, kind="ExternalOutput")
    with tile.TileContext(nc) as tc, contextlib.ExitStack() as ctx:
        _trace_kernel(
            ctx, tc, y_d, xb_d, xq_d, wq_d, wk_d, wv_d, bq_d, bk_d, bvb_d, g_d, out_d
        )
    nc.compile()
    _PROGRAM_CACHE["nc"] = nc
    return nc


def _make_in_maps(inputs):
    import ml_dtypes

    BF = ml_dtypes.bfloat16
    x = np.ascontiguousarray(inputs["x"], dtype=np.float32).reshape(B, C, N)
    y = np.asarray(inputs["y"], np.float32).reshape(B, C, N)
    y_bf = np.ascontiguousarray(y.astype(BF))
    wq_t = np.ascontiguousarray(
        np.asarray(inputs["Wq"], np.float32)
        .astype(BF).T.reshape(CCH, 128, CQK).transpose(1, 0, 2)
    )
    wk_t = np.ascontiguousarray(
        np.asarray(inputs["Wk"], np.float32)
        .astype(BF).T.reshape(CCH, 128, CQK).transpose(1, 0, 2)
    )
    wv_t = np.ascontiguousarray(
        np.asarray(inputs["Wv"], np.float32)
        .astype(BF).T.reshape(CCH, 128, C).transpose(1, 0, 2)
    )
    bq_c = np.ascontiguousarray(np.asarray(inputs["bq"], np.float32).reshape(CQK, 1))
    bk_c = np.ascontiguousarray(np.asarray(inputs["bk"], np.float32).reshape(CQK, 1))
    bv_bc = np.ascontiguousarray(
        np.broadcast_to(
            np.asarray(inputs["bv"], np.float32).reshape(1, C), (128, C)
        ).astype(BF)
    )
    gamma_b = np.full(
        (128, 1), float(np.asarray(inputs["gamma"]).reshape(-1)[0]), np.float32
    )

    in_maps = []
    for core in range(NCORES):
        b, h = divmod(core, 2)
        xl = x[b, :, h * NLOC : (h + 1) * NLOC]  # [C, NLOC]
        xb = np.ascontiguousarray(xl.astype(BF).reshape(CCH, 128, NLOC))
        xq = np.ascontiguousarray(xl.T.reshape(NQC, 128, C))
        in_maps.append(
            {
                "y2": np.ascontiguousarray(y_bf[b].reshape(CCH, 128, N)),
                "xb": xb,
                "xq": xq,
                "wq_t": wq_t,
                "wk_t": wk_t,
                "wv_t": wv_t,
                "bq_c": bq_c,
                "bk_c": bk_c,
                "bv_bc": bv_bc,
                "gamma_b": gamma_b,
            }
        )
    return in_maps


def _assemble(results):
    out = np.empty((B, C, N), np.float32)
    for core in range(NCORES):
        b, h = divmod(core, 2)
        oq = np.asarray(results[core]["out_q"], dtype=np.float32)  # [16,128,C]
        out[b, :, h * NLOC : (h + 1) * NLOC] = oq.reshape(NLOC, C).T
    return out.reshape(B, C, 64, 64)


def run(inputs, trace=False, **kwargs):
    """Run the kernel; returns (full_output, BassKernelResults)."""
    nc = _get_program()
    in_maps = _make_in_maps(inputs)
    res = run_bass_kernel_spmd(
        nc, in_maps, core_ids=list(range(NCORES)), trace=trace, **kwargs
    )
    return _assemble(res.results), res


def kernel(**inputs) -> np.ndarray:
    out, _ = run(inputs, trace=False)
    return out


# revision 9
# speedup vs baseline: 1.3700x; 1.0496x over previous
"""Trainium2 Bass kernel for nn_CrossAttention (B=4, C=256, N=64*64=4096, CQK=32).

Reference computation:
    q = Wq @ xf + bq          [B, N, 32]
    k = Wk @ yf + bk          [B, 32, N]
    v = Wv @ yf + bv          [B, 256, N]
    attn = softmax(q @ k)     [B, N, N]
    out = gamma * (v @ attn^T) + x

Sharding: 8 cores = batch(4) x query-half(2). Each core owns 2048 query
positions of one sample and all 4096 keys of that sample.

v3 design notes:
  - Energy matmuls are full-array K=128 (kT/qT zero-padded 32->128).
    Row/col-masked tile_position matmuls are NOT counted as busy by the
    PE HAM activity monitor, which re-throttles the clock to 1.2GHz (v2
    measured 51% of the kernel at K=4/8) - so no PE tiling tricks.
    The zero rows are written once by two vector-engine memsets in the
    prologue (the old gpsimd memsets took 10us and gated the first MM).
  - exp granularity 1024: one ACT instruction per key-chunk pair reads a
    2-bank PSUM tile [128,2,512] and writes the fp8 ex tile. The scalar
    engine is the kernel's bottleneck; its ~150ns per-instruction overhead
    is paid half as often as with 512-wide activations.
  - AV uses fp8 DoubleRow (ex [128,2,512] stationary slices, vaug moving,
    pouts [q,272] with a ones-column producing the softmax denominator in
    col 256). bv is folded into vaug via a broadcast row tile.
  - Output is written q-major ([qchunk,128,C] bf16); the host transposes
    back to [C, N] fp32. This removes all PE transposes from the kernel.
  - vaug pairs 8-15 + kT slices 3-7 build inside quarter 0 so the PE
    prologue overlaps the ACT-bound attention stream.
  - DMA only on the sync + gpsimd queues (the scalar queue must stay free
    for the exp activations; a dummy exp warms the ACT table early).
"""

import contextlib

import numpy as np

import concourse.mybir as mybir
import concourse.tile as tile
from concourse import bacc
from concourse.bass_utils import run_bass_kernel_spmd

F32 = mybir.dt.float32
F8 = mybir.dt.float8e4
BF16 = mybir.dt.bfloat16
AFT = mybir.ActivationFunctionType
DR = mybir.MatmulPerfMode.DoubleRow

B = 4
C = 256
CQK = 32
N = 4096  # 64 * 64
NCORES = 8
NLOC = N // 2  # 2048 queries per core
CCH = C // 128  # 2 channel chunks
MC = N // 128  # 32 key chunks
NPAIR = MC // 2  # 16 key-chunk pairs
NQ = 4  # query quarters per core
QW = NLOC // NQ  # 512
NQC = NLOC // 128  # 16 query chunks of 128
VW = 272  # vaug width: 256 v channels + denominator col + pad to step%16==0


def _trace_kernel(ctx, tc, y_d, xb_d, xq_d, wq_d, wk_d, wv_d, bq_d, bk_d, bvb_d, g_d, out_d):
    nc = tc.nc

    const = ctx.enter_context(tc.tile_pool(name="const", bufs=1))
    big = ctx.enter_context(tc.tile_pool(name="big", bufs=1))
    vaugp = ctx.enter_context(tc.tile_pool(name="vaugp", bufs=NPAIR))
    expp = ctx.enter_context(tc.tile_pool(name="expp", bufs=4))
    finp = ctx.enter_context(tc.tile_pool(name="finp", bufs=4))
    smallp = ctx.enter_context(tc.tile_pool(name="smallp", bufs=8))
    # PSUM (8 banks): pep = 2 x [128,2,512] f32 (2 banks each, shared by the
    # energy pairs and, as rotation passengers, the projection matmuls);
    # poutp = 4 x [128,272] f32 (1 bank each) AV accumulators.
    pep = ctx.enter_context(tc.tile_pool(name="pep", bufs=2, space="PSUM"))
    poutp = ctx.enter_context(tc.tile_pool(name="poutp", bufs=4, space="PSUM"))

    # ---- tiny consts + ACT exp table warm-up (before anything else on ACT)
    scr = const.tile([1, 1], F32, tag="scr")
    nc.vector.memset(scr, 0.0)
    nc.scalar.activation(scr, scr, AFT.Exp)

    onep_sb = const.tile([128, VW - C], F8, tag="onep_sb")
    nc.vector.memset(onep_sb, 0.0)
    nc.vector.memset(onep_sb[:, 0:1], 1.0)

    # padded projection outputs; rows 32:128 stay zero so the energy matmul
    # can contract over the full 128 partitions (full-array MMs keep HAM warm
    # and get the fast weight load).
    kT_sb = big.tile([128, N], BF16, tag="kT_sb")
    nc.vector.memset(kT_sb, 0.0)
    qT_sb = big.tile([128, NLOC], BF16, tag="qT_sb")
    nc.vector.memset(qT_sb, 0.0)

    # ---- DMAs. Each DMA_DIRECT2D occupies its issuing queue ~630ns, so the
    # sync-queue order is chosen to land (wk, wq, y slice 0) as early as
    # possible — they gate the first energy pair and thus the exp stream.
    wk_b = const.tile([128, CCH, CQK], BF16, tag="wk_b")
    nc.sync.dma_start(out=wk_b, in_=wk_d.ap())
    wq_b = const.tile([128, CCH, CQK], BF16, tag="wq_b")
    nc.sync.dma_start(out=wq_b, in_=wq_d.ap())
    y_sb = big.tile([128, CCH, N], BF16, tag="y_sb")

    def y_piece(s):
        sl = slice(s * QW, (s + 1) * QW)
        for cc in range(CCH):
            nc.sync.dma_start(out=y_sb[:, cc, sl], in_=y_d.ap()[cc, :, sl])

    y_piece(0)
    bq_sb = const.tile([CQK, 1], F32, tag="bq_sb")
    nc.sync.dma_start(out=bq_sb, in_=bq_d.ap())
    bk_sb = const.tile([CQK, 1], F32, tag="bk_sb")
    nc.sync.dma_start(out=bk_sb, in_=bk_d.ap())
    g_sb = const.tile([128, 1], F32, tag="g_sb")
    nc.sync.dma_start(out=g_sb, in_=g_d.ap())
    wv_b = const.tile([128, CCH, C], BF16, tag="wv_b")
    nc.sync.dma_start(out=wv_b, in_=wv_d.ap())
    bvb_sb = const.tile([128, C], BF16, tag="bvb_sb")
    nc.sync.dma_start(out=bvb_sb, in_=bvb_d.ap())
    for s in range(1, 8):
        y_piece(s)
    # x tensors on the gpsimd queue (gpsimd is otherwise idle).
    xb_sb = big.tile([128, CCH, NLOC], BF16, tag="xb_sb")
    for cc in range(CCH):
        nc.gpsimd.dma_start(out=xb_sb[:, cc, :], in_=xb_d.ap()[cc])
    xq_sb = big.tile([128, NQC, C], F32, tag="xq_sb")
    nc.gpsimd.dma_start(out=xq_sb, in_=xq_d.ap())

    # ---- projections (baseline-style full-contraction matmuls) -----------
    def kT_slice(s):
        # keys [512s, 512s+512)
        sl = slice(s * QW, (s + 1) * QW)
        pk = pep.tile([CQK, QW], F32, tag="pe", name=f"pk{s}")
        for cc in range(CCH):
            nc.tensor.matmul(
                pk,
                lhsT=wk_b[:, cc, :],
                rhs=y_sb[:, cc, sl],
                start=(cc == 0),
                stop=(cc == CCH - 1),
            )
        nc.vector.tensor_scalar_add(kT_sb[0:CQK, sl], pk, bk_sb)

    def qT_slice(s):
        sl = slice(s * QW, (s + 1) * QW)
        pq = pep.tile([CQK, QW], F32, tag="pe", name=f"pq{s}")
        for cc in range(CCH):
            nc.tensor.matmul(
                pq,
                lhsT=wq_b[:, cc, :],
                rhs=xb_sb[:, cc, sl],
                start=(cc == 0),
                stop=(cc == CCH - 1),
            )
        nc.vector.tensor_scalar_add(qT_sb[0:CQK, sl], pq, bq_sb)

    vaug = []

    def vaug_build(g):
        va = vaugp.tile([128, 2, VW], F8, tag="vaug", name=f"vaug{g}")
        for j in range(2):
            mc = 2 * g + j
            pv = pep.tile([128, C], F32, tag="pe", name=f"pv{mc}")
            for cc in range(CCH):
                nc.tensor.matmul(
                    pv,
                    lhsT=y_sb[:, cc, mc * 128 : (mc + 1) * 128],
                    rhs=wv_b[:, cc, :],
                    start=(cc == 0),
                    stop=(cc == CCH - 1),
                )
            nc.vector.tensor_add(va[:, j, :C], pv, bvb_sb)
            nc.vector.tensor_copy(va[:, j, C:VW], onep_sb)
        vaug.append(va)

    # prologue: just enough to start the exp stream; everything else weaves
    # into quarter 0 behind the first energy pair.
    kT_slice(0)
    qT_slice(0)
    vaug_build(0)

    # ---- attention quarters --------------------------------------------
    for qt in range(NQ):
        nsl = slice(qt * QW, (qt + 1) * QW)
        pouts = [
            poutp.tile([128, VW], F32, tag="pout", name=f"pout{qt}_{i}")
            for i in range(4)
        ]

        def do_av(g, ex):
            for ncc in range(4):
                nc.tensor.matmul(
                    pouts[ncc],
                    lhsT=ex[:, :, ncc * 128 : (ncc + 1) * 128],
                    rhs=vaug[g],
                    start=(g == 0),
                    stop=(g == NPAIR - 1),
                    perf_mode=DR,
                )

        pending = []
        for g in range(NPAIR):
            pe_t = pep.tile([128, 2, QW], F32, tag="pe", name=f"pe{qt}_{g}")
            for j in range(2):
                c = 2 * g + j
                nc.tensor.matmul(
                    pe_t[:, j, :],
                    lhsT=kT_sb[:, c * 128 : (c + 1) * 128],
                    rhs=qT_sb[:, nsl],
                    start=True,
                    stop=True,
                )
            ex = expp.tile([128, 2, QW], F8, tag="exp", name=f"ex{qt}_{g}")
            nc.scalar.activation(ex, pe_t, AFT.Exp)
            if qt == 0:
                # weave the remaining projections + vaug builds behind the
                # energy/exp stream (they fill the PE while ACT works).
                if g % 2 == 0 and g // 2 + 1 <= 7:
                    kT_slice(g // 2 + 1)
                if g + 1 < NPAIR:
                    vaug_build(g + 1)
                if g in (3, 6, 9):
                    qT_slice(g // 3)
            # AV lags two pairs so its LDWEIGHTS never waits on an exp.
            pending.append((g, ex))
            if len(pending) > 2:
                do_av(*pending.pop(0))
        for item in pending:
            do_av(*item)

        # drain: normalize by the denominator column, scale by gamma, add the
        # (host-pretransposed) residual, DMA out q-major bf16.
        for ncc in range(4):
            po = pouts[ncc]
            qidx = qt * 4 + ncc
            rec = smallp.tile([128, 1], F32, tag="rec", name=f"rec{qidx}")
            nc.vector.reciprocal(rec, po[:, C : C + 1])
            grec = smallp.tile([128, 1], F32, tag="grec", name=f"grec{qidx}")
            nc.vector.tensor_scalar_mul(grec, rec, g_sb)
            tmp = finp.tile([128, C], BF16, tag="tmp", name=f"tmp{qidx}")
            nc.vector.tensor_scalar_mul(tmp, po[:, :C], grec)
            fin = finp.tile([128, C], BF16, tag="fin", name=f"fin{qidx}")
            nc.vector.tensor_add(fin, tmp, xq_sb[:, qidx, :])
            nc.gpsimd.dma_start(out=out_d.ap()[qidx], in_=fin)


_PROGRAM_CACHE = {}


def _get_program():
    if "nc" in _PROGRAM_CACHE:
        return _PROGRAM_CACHE["nc"]
    nc = bacc.Bacc("TRN2", target_bir_lowering=False, debug=False)
    y_d = nc.dram_tensor("y2", [CCH, 128, N], BF16, kind="ExternalInput")
    xb_d = nc.dram_tensor("xb", [CCH, 128, NLOC], BF16, kind="ExternalInput")
    xq_d = nc.dram_tensor("xq", [128, NQC, C], F32, kind="ExternalInput")
    wq_d = nc.dram_tensor("wq_t", [128, CCH, CQK], BF16, kind="ExternalInput")
    wk_d = nc.dram_tensor("wk_t", [128, CCH, CQK], BF16, kind="ExternalInput")
    wv_d = nc.dram_tensor("wv_t", [128, CCH, C], BF16, kind="ExternalInput")
    bq_d = nc.dram_tensor("bq_c", [CQK, 1], F32, kind="ExternalInput")
    bk_d = nc.dram_tensor("bk_c", [CQK, 1], F32, kind="ExternalInput")
    bvb_d = nc.dram_tensor("bv_bc", [128, C], BF16, kind="ExternalInput")
    g_d = nc.dram_tensor("gamma_b", [128, 1], F32, kind="ExternalInput")
    out_d = nc.dram_tensor("out_q", [NQC, 128, C], BF16, kind="ExternalOutput")
    with tile.TileContext(nc) as tc, contextlib.ExitStack() as ctx:
        _trace_kernel(
            ctx, tc, y_d, xb_d, xq_d, wq_d, wk_d, wv_d, bq_d, bk_d, bvb_d, g_d, out_d
        )
    nc.compile()
    _PROGRAM_CACHE["nc"] = nc
    return nc


def _make_in_maps(inputs):
    import ml_dtypes

    BF = ml_dtypes.bfloat16
    x = np.ascontiguousarray(inputs["x"], dtype=np.float32).reshape(B, C, N)
    y = np.asarray(inputs["y"], np.float32).reshape(B, C, N)
    y_bf = np.ascontiguousarray(y.astype(BF))
    wq_t = np.ascontiguousarray(
        np.asarray(inputs["Wq"], np.float32)
        .astype(BF).T.reshape(CCH, 128, CQK).transpose(1, 0, 2)
    )
    wk_t = np.ascontiguousarray(
        np.asarray(inputs["Wk"], np.float32)
        .astype(BF).T.reshape(CCH, 128, CQK).transpose(1, 0, 2)
    )
    wv_t = np.ascontiguousarray(
        np.asarray(inputs["Wv"], np.float32)
        .astype(BF).T.reshape(CCH, 128, C).transpose(1, 0, 2)
    )
    bq_c = np.ascontiguousarray(np.asarray(inputs["bq"], np.float32).reshape(CQK, 1))
    bk_c = np.ascontiguousarray(np.asarray(inputs["bk"], np.float32).reshape(CQK, 1))
    bv_bc = np.ascontiguousarray(
        np.broadcast_to(
            np.asarray(inputs["bv"], np.float32).reshape(1, C), (128, C)
        ).astype(BF)
    )
    gamma_b = np.full(
        (128, 1), float(np.asarray(inputs["gamma"]).reshape(-1)[0]), np.float32
    )

    in_maps = []
    for core in range(NCORES):
        b, h = divmod(core, 2)
        xl = x[b, :, h * NLOC : (h + 1) * NLOC]  # [C, NLOC]
        xb = np.ascontiguousarray(xl.astype(BF).reshape(CCH, 128, NLOC))
        xq = np.ascontiguousarray(xl.T.reshape(NQC, 128, C).transpose(1, 0, 2))
        in_maps.append(
            {
                "y2": np.ascontiguousarray(y_bf[b].reshape(CCH, 128, N)),
                "xb": xb,
                "xq": xq,
                "wq_t": wq_t,
                "wk_t": wk_t,
                "wv_t": wv_t,
                "bq_c": bq_c,
                "bk_c": bk_c,
                "bv_bc": bv_bc,
                "gamma_b": gamma_b,
            }
        )
    return in_maps


def _assemble(results):
    out = np.empty((B, C, N), np.float32)
    for core in range(NCORES):
        b, h = divmod(core, 2)
        oq = np.asarray(results[core]["out_q"], dtype=np.float32)  # [16,128,C]
        out[b, :, h * NLOC : (h + 1) * NLOC] = oq.reshape(NLOC, C).T
    return out.reshape(B, C, 64, 64)


def run(inputs, trace=False, **kwargs):
    """Run the kernel; returns (full_output, BassKernelResults)."""
    nc = _get_program()
    in_maps = _make_in_maps(inputs)
    res = run_bass_kernel_spmd(
        nc, in_maps, core_ids=list(range(NCORES)), trace=trace, **kwargs
    )
    return _assemble(res.results), res


def kernel(**inputs) -> np.ndarray:
    out, _ = run(inputs, trace=False)
    return out
